# revision 2
# baseline (speedup 1.0000x reference)
"""V3: v2 + aA sample-parity split (no duplicated aA copy), pair-packed
transpose epilogue, dneg/norm muls on gpsimd, one-time pad memsets."""

import numpy as np
from contextlib import ExitStack

import concourse.bass as bass
import concourse.bacc as bacc
import concourse.tile as tile
from concourse import mybir
from concourse import bass_utils

F32 = mybir.dt.float32
F16 = mybir.dt.float16
AX = mybir.AxisListType
OP = mybir.AluOpType
AF = mybir.ActivationFunctionType

B, N, C = 2048, 62, 512
L, H, Co = 3, 8, 64
NP = 64
NCORES = 8
BC = B // NCORES
TILE_B = 8
KC = C // 128
HP = H // 2
HCo = H * Co


def make_identity_f32(nc, identity):
    nc.gpsimd.memset(identity, 0.0)
    nc.gpsimd.affine_select(
        out=identity, in_=identity,
        compare_op=OP.not_equal, fill=1.0, base=0,
        pattern=[[-1, identity.shape[0]]], channel_multiplier=1,
    )


def build_program(bc: int, repeat: int = 1):
    nt = bc // TILE_B
    nc = bacc.Bacc("TRN2", target_bir_lowering=False, debug=False)

    x_d = nc.dram_tensor("x", [bc, N, C], F32, kind="ExternalInput").ap()
    wa_d = nc.dram_tensor("wa_h", [128, L, HP, KC, 128], F16, kind="ExternalInput").ap()
    w_d = nc.dram_tensor("w_h", [128, L, KC, HCo], F16, kind="ExternalInput").ap()
    ah_d = nc.dram_tensor("ah_h", [128, L, 128], F16, kind="ExternalInput").ap()
    dn_d = nc.dram_tensor("dn_h", [L, NP], F16, kind="ExternalInput").ap()
    out_d = nc.dram_tensor("out", [bc, N, HCo], F32, kind="ExternalOutput").ap()

    with tile.TileContext(nc) as tc, ExitStack() as ctx:
        statics = ctx.enter_context(tc.tile_pool(name="statics", bufs=1))
        wa_sb = statics.tile([128, L, HP, KC, 128], F16)
        nc.sync.dma_start(out=wa_sb, in_=wa_d)
        w_sb = statics.tile([128, L, KC, HCo], F16)
        nc.sync.dma_start(out=w_sb, in_=w_d)
        ah_sb = statics.tile([128, L, 128], F16)
        nc.sync.dma_start(out=ah_sb, in_=ah_d)
        dn_sb = statics.tile([128, L, NP], F16)
        dn_src = bass.AP(
            tensor=dn_d.tensor, offset=dn_d.offset,
            ap=[[0, 128], [1, L * NP]],
        )
        nc.sync.dma_start(out=dn_sb, in_=dn_src)
        ident = statics.tile([128, 128], F32)
        make_identity_f32(nc, ident[:])

        xp = ctx.enter_context(tc.tile_pool(name="xp", bufs=2))
        xtp = ctx.enter_context(tc.tile_pool(name="xtp", bufs=2))
        xtlp = ctx.enter_context(tc.tile_pool(name="xtlp", bufs=2))
        atp = ctx.enter_context(tc.tile_pool(name="atp", bufs=2))
        e2p = ctx.enter_context(tc.tile_pool(name="e2p", bufs=3))
        dnp = ctx.enter_context(tc.tile_pool(name="dnp", bufs=3))
        ubf = ctx.enter_context(tc.tile_pool(name="ubf", bufs=3))
        aabf = ctx.enter_context(tc.tile_pool(name="aabf", bufs=2))
        accp = ctx.enter_context(tc.tile_pool(name="accp", bufs=3))
        outp = ctx.enter_context(tc.tile_pool(name="outp", bufs=2))
        ps = ctx.enter_context(tc.tile_pool(name="ps", bufs=3, space="PSUM"))
        psu = ctx.enter_context(tc.tile_pool(name="psu", bufs=1, space="PSUM"))
        psf = ctx.enter_context(tc.tile_pool(name="psf", bufs=2, space="PSUM"))
        psa = ctx.enter_context(tc.tile_pool(name="psa", bufs=1, space="PSUM"))

        for t in range(nt * repeat):
            it = t
            t = t % nt
            b0 = t * TILE_B
            abf_tiles = {}
            acc_tiles = {}
            x_nat = xp.tile([N, TILE_B, C], F32, tag="x")
            nc.sync.dma_start(
                out=x_nat, in_=x_d[b0 : b0 + TILE_B].rearrange("b n c -> n b c")
            )
            obuf = outp.tile([128, TILE_B // 2, HCo], F32, tag="obuf")

            # ---- transpose to xT[c, kc, b, np] fp16; pads zeroed on first use
            xT = xtp.tile([128, KC, TILE_B, NP], F16, tag="xT")
            if it < 2:
                nc.vector.memset(xT[:, :, :, N:NP], 0.0)
            for q in range(TILE_B // 2):
                pt = ps.tile([128, 2, KC, N], F32, tag="lg")
                for j in range(2):
                    for kc in range(KC):
                        nc.tensor.transpose(
                            pt[:, j, kc],
                            x_nat[:, 2 * q + j, kc * 128 : (kc + 1) * 128],
                            ident[:N, :N],
                        )
                nc.scalar.copy(
                    out=xT[:, :, 2 * q : 2 * q + 2, 0:N],
                    in_=pt.rearrange("p j k n -> p k j n"),
                )

            for l in range(L):
                if l == 0:
                    xTl = xT
                else:
                    xTl = xtlp.tile([128, KC, TILE_B, NP], F16, tag="xTl")
                    dnl = dn_sb[:, l]
                    dn_b = bass.AP(
                        tensor=dnl.tensor, offset=dnl.offset,
                        ap=[dnl.ap[0], [0, TILE_B], dnl.ap[1]],
                    )
                    for kc in range(KC):
                        nc.gpsimd.tensor_mul(xTl[:, kc], xT[:, kc], dn_b)

                for hp in range(HP):
                    zp = ps.tile([128, TILE_B, NP], F32, tag="lg")
                    for kc in range(KC):
                        nc.tensor.matmul(
                            zp,
                            lhsT=wa_sb[:, l, hp, kc],
                            rhs=xTl[:, kc],
                            start=(kc == 0),
                            stop=(kc == KC - 1),
                        )

                    # ---- s = exp(leaky_relu(z)); softmax over n
                    s = atp.tile([128, TILE_B, NP], F16, tag=f"aT_{hp}")
                    e2 = e2p.tile([128, TILE_B, NP], F16, tag="aT2")
                    nc.scalar.activation(out=e2, in_=zp, func=AF.Prelu, alpha=0.01)
                    nc.scalar.activation(out=s, in_=e2, func=AF.Exp)
                    den = dnp.tile([128, TILE_B], F32, tag="den")
                    nc.vector.reduce_sum(out=den, in_=s[:, :, 0:N], axis=AX.X)
                    rden = dnp.tile([128, TILE_B], F32, tag="rden")
                    nc.vector.reciprocal(rden, den)
                    rb = bass.AP(
                        tensor=rden.tensor,
                        offset=rden.offset,
                        ap=[rden.ap[0], rden.ap[1], [0, N]],
                    )
                    nc.vector.tensor_mul(s[:, :, 0:N], s[:, :, 0:N], rb)

                    # ---- aA = (a @ A_hat), duplicated layout (v2 structure)
                    pa = psa.tile([128, 2, TILE_B, NP], F32, tag="aA")
                    for par in range(2):
                        hb = 64 * par
                        nc.tensor.matmul(
                            pa[:, par],
                            lhsT=ah_sb[hb : hb + N, l],
                            rhs=s[hb : hb + N],
                            start=True,
                            stop=True,
                        )
                    abf = aabf.tile([128, 2, TILE_B, NP], F16, tag=f"aA_{hp}")
                    nc.scalar.copy(out=abf, in_=pa)
                    abf_tiles[(l, hp)] = abf

                for pi in range(TILE_B // 2):
                    up = psu.tile([128, H, Co], F32, tag="u")
                    for kc in range(KC):
                        nc.tensor.matmul(
                            up,
                            lhsT=xT[:, kc, 2 * pi : 2 * pi + 2],
                            rhs=w_sb[:, l, kc],
                            start=(kc == 0),
                            stop=(kc == KC - 1),
                        )
                    ub = ubf.tile([128, H, Co], F16, tag="u")
                    nc.vector.tensor_copy(out=ub, in_=up)

                    fp = psf.tile([128, H, Co], F32, tag="fin")
                    for h in range(H):
                        abf_t = abf_tiles[(l, h // 2)]
                        for sp in range(2):
                            rb0 = 64 * sp
                            bloc = 2 * pi + sp
                            nc.tensor.matmul(
                                fp[rb0 : rb0 + NP, h],
                                lhsT=abf_t[rb0 : rb0 + N, h % 2, bloc, 0:NP],
                                rhs=ub[rb0 : rb0 + N, h],
                                start=True,
                                stop=True,
                                tile_position=(rb0, rb0),
                            )
                    if l == 0:
                        nacc = accp.tile([128, H, Co], F32, tag=f"acc_{pi}")
                        nc.vector.tensor_scalar_max(nacc, fp, 0.0)
                        acc_tiles[pi] = nacc
                    elif l < L - 1:
                        nacc = accp.tile([128, H, Co], F32, tag=f"acc_{pi}")
                        nc.vector.scalar_tensor_tensor(
                            out=nacc, in0=fp, scalar=0.0, in1=acc_tiles[pi],
                            op0=OP.max, op1=OP.add,
                        )
                        acc_tiles[pi] = nacc
                    else:
                        ob = obuf[:, pi].rearrange("p (h o) -> p h o", h=H)
                        nc.vector.scalar_tensor_tensor(
                            out=ob, in0=fp, scalar=0.0, in1=acc_tiles[pi],
                            op0=OP.max, op1=OP.add,
                        )

            for sp in range(2):
                src = obuf[64 * sp : 64 * sp + N]
                dst = bass.AP(
                    tensor=out_d.tensor,
                    offset=out_d.offset + (b0 + sp) * N * HCo,
                    ap=[[HCo, N], [2 * N * HCo, TILE_B // 2], [1, HCo]],
                )
                nc.sync.dma_start(out=dst, in_=src)
    nc.finalize()
    return nc


def pack_weights(Lap, W_alphas, W):
    I = np.eye(N, dtype=np.float32)
    adjs = [I, Lap, Lap @ Lap]
    wa_pack = np.zeros((L, HP, KC, 128, 128), np.float16)
    w_flat = np.zeros((L, KC, 128, HCo), np.float16)
    ah_dup = np.zeros((L, 128, 128), np.float16)
    dneg_pad = np.zeros((L, NP), np.float16)
    for l in range(L):
        A = adjs[l]
        A_hat = (A + I).astype(np.float16)
        D = A.sum(-1)
        dneg_pad[l, :N] = np.where(D == 0, 0.0, 1.0 / D).astype(np.float16)
        for q in (0, 64):
            ah_dup[l, 0:N, q : q + N] = A_hat
            ah_dup[l, 64 : 64 + N, q : q + N] = A_hat
        for hp in range(HP):
            for kc in range(KC):
                wa_pack[l, hp, kc, :, 0:N] = W_alphas[l, 2 * hp, kc * 128 : (kc + 1) * 128, :]
                wa_pack[l, hp, kc, :, 64 : 64 + N] = W_alphas[l, 2 * hp + 1, kc * 128 : (kc + 1) * 128, :]
        for kc in range(KC):
            for h in range(H):
                w_flat[l, kc, :, h * Co : (h + 1) * Co] = W[l, h, kc * 128 : (kc + 1) * 128, :]
    wa_h = np.ascontiguousarray(wa_pack.transpose(3, 0, 1, 2, 4))
    w_h = np.ascontiguousarray(w_flat.transpose(2, 0, 1, 3))
    ah_h = np.ascontiguousarray(ah_dup.transpose(1, 0, 2))
    return wa_h, w_h, ah_h, dneg_pad


_CACHED = {}


def kernel(x, L_mat=None, **kw):
    if L_mat is None:
        L_mat = kw.pop("L")
    W_alphas = kw.pop("W_alphas")
    W = kw.pop("W")
    x = np.ascontiguousarray(np.asarray(x, np.float32))
    L_mat = np.asarray(L_mat, np.float32)
    W_alphas = np.asarray(W_alphas, np.float32)
    W = np.asarray(W, np.float32)

    wa_h, w_h, ah_h, dn_h = pack_weights(L_mat, W_alphas, W)

    if "nc" not in _CACHED:
        _CACHED["nc"] = build_program(BC)
    nc = _CACHED["nc"]

    in_maps = []
    for c in range(NCORES):
        in_maps.append(
            {
                "x": x[c * BC : (c + 1) * BC],
                "wa_h": wa_h,
                "w_h": w_h,
                "ah_h": ah_h,
                "dn_h": dn_h,
            }
        )
    res = bass_utils.run_bass_kernel_spmd(nc, in_maps, core_ids=list(range(NCORES)))
    out = np.concatenate([r["out"] for r in res.results], axis=0)
    return out.reshape(B, N, HCo)
